# revision 15
# baseline (speedup 1.0000x reference)
"""Trainium2 Bass kernel for nn_MessagePassing (gnn_message_passing).

Math (per batch b):
    coef[s,e] = sum_o adj[s,o] * edge[s,o,e]
    v[s,e,i]  = sum_j W[e,i,j] * node[s,j]
    out[s,i]  = sum_e coef[s,e] * v[s,e,i]

Sharding: data parallel over the batch axis - core b handles batch b.

Host-side staging (per core):
  * edge  -> [t, o%128, o//128, e, s%128] bf16: o on SBUF partitions so the
    o-reduction runs on the PE; contiguous 2 MiB DMA per s-tile.
  * adj   -> [o%128, t, o//128, s%128] bf16 (same partition layout).
  * node  -> nodeT [j, s] bf16, W -> wT [j, e, i] bf16.
  * out   <- [p, t, i] f32, one contiguous DMA.

Engine assignment per s-tile (measured HW rates):
  * DVE: 8 2x-TT multiplies prod[o, e, s] = edge * adj (adj broadcast over
    the middle e dim; bf16-packed 2x mode, ~0.54 ns/elem) and one 2x-
    ineligible TT (PSUM operand) forming uT[j,e,s] = nodeT[j,s]*coef[s,e].
  * PE : the ENTIRE reduction: coefRow[1,(e,s)] = sum_o ones^T @ prod
    accumulated over the 8 o-blocks in PSUM, then 2 ones-matmuls that
    broadcast coefRow across partitions, then 8 PSUM-accumulated matmuls
    out[s,i] += uT_e^T @ W_e.
  * ACT: small copies only (coefRow PSUM->SBUF, out PSUM->SBUF).
  * Pool shares the DVE SBUF port - unused.
"""

import numpy as np
import ml_dtypes
from contextlib import ExitStack

import concourse.bass as bass
import concourse.bacc as bacc
import concourse.mybir as mybir
import concourse.tile as tile
from concourse.bass_utils import run_bass_kernel_spmd

B, N, D, E = 8, 1024, 128, 8
P = 128
NT = N // P  # 8 s-tiles per core
OB = N // P  # 8 o-blocks
HF = E * P // 2  # 512: half of the flattened (e, s) row

F32 = mybir.dt.float32
BF16 = mybir.dt.bfloat16
MUL = mybir.AluOpType.mult
COPY = mybir.ActivationFunctionType.Copy

BF16_NP = ml_dtypes.bfloat16


def build_nc():
    nc = bacc.Bacc("TRN2", target_bir_lowering=False, debug=False, num_devices=B)

    edge_d = nc.dram_tensor("edge_t", [NT, P, OB, E, P], BF16, kind="ExternalInput").ap()
    adj_d = nc.dram_tensor("adj_r", [P, NT, OB, P], BF16, kind="ExternalInput").ap()
    nodeT_d = nc.dram_tensor("nodeT", [D, N], BF16, kind="ExternalInput").ap()
    wT_d = nc.dram_tensor("wT", [D, E, D], BF16, kind="ExternalInput").ap()
    out_d = nc.dram_tensor("out", [P, NT, D], F32, kind="ExternalOutput").ap()

    with tile.TileContext(nc) as tc, ExitStack() as ctx:
        const_pool = ctx.enter_context(tc.tile_pool(name="const", bufs=1))
        edge_pool = ctx.enter_context(tc.tile_pool(name="edge", bufs=4))
        prod_pool = ctx.enter_context(tc.tile_pool(name="prod", bufs=2))
        work_pool = ctx.enter_context(tc.tile_pool(name="work", bufs=2))
        ps_row_pool = ctx.enter_context(tc.tile_pool(name="psr", bufs=2, space="PSUM"))
        ps_bc_pool = ctx.enter_context(tc.tile_pool(name="psb", bufs=1, space="PSUM"))
        ps_out_pool = ctx.enter_context(tc.tile_pool(name="pso", bufs=2, space="PSUM"))

        adj_all = const_pool.tile([P, NT, OB, P], BF16)
        nodeT = const_pool.tile([P, N], BF16)
        wT = const_pool.tile([P, E, D], BF16)
        ones_r = const_pool.tile([P, 1], BF16)  # reduce lhsT
        ones_b = const_pool.tile([1, P], BF16)  # broadcast lhsT
        acc_all = const_pool.tile([P, NT, D], F32)

        nc.vector.memset(ones_r[:], 1.0)
        nc.vector.memset(ones_b[:], 1.0)

        # Edge stream on the sync queue (tile 0 split for a fast start);
        # everything else on the scalar queue.
        def load_edge(t, split=False):
            et = edge_pool.tile([P, OB, E, P], BF16, tag="edge_t")
            if split:
                nc.sync.dma_start(et[:, 0:1, :, :], edge_d[t, :, 0:1])
                nc.sync.dma_start(et[:, 1:3, :, :], edge_d[t, :, 1:3])
                nc.sync.dma_start(et[:, 3:OB, :, :], edge_d[t, :, 3:OB])
            else:
                nc.sync.dma_start(et[:], edge_d[t])
            return et

        edge_tiles = {0: load_edge(0, split=True)}
        nc.scalar.dma_start(adj_all[:, 0, :, :], adj_d[:, 0, :, :])
        nc.scalar.dma_start(nodeT[:], nodeT_d)
        nc.scalar.dma_start(wT[:], wT_d)
        for t in range(1, NT):
            edge_tiles[t] = load_edge(t)
            nc.scalar.dma_start(adj_all[:, t, :, :], adj_d[:, t, :, :])

        for t in range(NT):
            edge_t = edge_tiles[t]

            # DVE: prod[o, e, s] = edge * adj (2x TT, adj bcast over e) and
            # PE: coefRow[1, (e,s)] += ones^T @ prod, per o-block.
            prod = prod_pool.tile([P, OB, E, P], BF16)
            rowA = ps_row_pool.tile([1, HF], F32, tag="rowA")
            rowB = ps_row_pool.tile([1, HF], F32, tag="rowB")
            rows = [rowA, rowB]
            nc.vector.tensor_tensor(
                out=prod[:],
                in0=edge_t[:],
                in1=adj_all[:, t, :, :][:, :, None, :].broadcast_to((P, OB, E, P)),
                op=MUL,
            )
            for ob in range(OB):
                flat = prod[:, ob, :, :].rearrange("p e s -> p (e s)")
                for h in range(2):
                    nc.tensor.matmul(
                        rows[h][:],
                        lhsT=ones_r[:],
                        rhs=flat[:, h * HF : (h + 1) * HF],
                        start=(ob == 0),
                        stop=(ob == OB - 1),
                    )

            # ACT: coefRow PSUM -> SBUF bf16
            coefRow = work_pool.tile([1, E * P], BF16)
            for h in range(2):
                nc.scalar.copy(coefRow[:, h * HF : (h + 1) * HF], rows[h][:])

            # PE: broadcast coefRow across 128 partitions
            bcA = ps_bc_pool.tile([P, HF], F32, tag="bcA")
            bcB = ps_bc_pool.tile([P, HF], F32, tag="bcB")
            coefB = [bcA, bcB]
            for h in range(2):
                nc.tensor.matmul(
                    coefB[h][:],
                    lhsT=ones_b[:],
                    rhs=coefRow[:, h * HF : (h + 1) * HF],
                    start=True,
                    stop=True,
                )

            # DVE: uT[j, e, s] = nodeT[j, s] * coef[s, e]
            uT = work_pool.tile([P, E, P], BF16)
            for h in range(2):
                nc.vector.tensor_tensor(
                    out=uT[:, h * 4 : (h + 1) * 4, :],
                    in0=nodeT[:, bass.ts(t, P)][:, None, :].broadcast_to((P, 4, P)),
                    in1=coefB[h][:].rearrange("p (e s) -> p e s", e=4),
                    op=MUL,
                )

            # PE: out[s, i] = sum_e uT_e^T @ W_e, accumulated in PSUM
            po = ps_out_pool.tile([P, D], F32, tag="po")
            for e in range(E):
                nc.tensor.matmul(
                    po[:], lhsT=uT[:, e, :], rhs=wT[:, e, :],
                    start=(e == 0), stop=(e == E - 1),
                )
            # ACT: out PSUM -> SBUF, then stream out per tile
            nc.scalar.copy(acc_all[:, t, :], po[:])
            nc.sync.dma_start(out_d[:, t, :], acc_all[:, t, :])

    nc.compile()
    return nc


_NC_CACHE = None


def get_nc():
    global _NC_CACHE
    if _NC_CACHE is None:
        _NC_CACHE = build_nc()
    return _NC_CACHE


def make_in_maps(node_state, edge_type_mat, adj_mat, W):
    node_state = np.asarray(node_state, dtype=np.float32)
    edge_type_mat = np.asarray(edge_type_mat, dtype=np.float32)
    adj_mat = np.asarray(adj_mat, dtype=np.float32)
    W = np.asarray(W, dtype=np.float32)

    wT = np.ascontiguousarray(W.transpose(2, 0, 1)).astype(BF16_NP)  # [j, e, i]
    in_maps = []
    for b in range(B):
        edge16 = edge_type_mat[b].astype(BF16_NP)  # [s, o, e]
        # [t, po, ob, e, ps]
        edge_t = np.ascontiguousarray(
            edge16.reshape(NT, P, OB, P, E).transpose(0, 3, 2, 4, 1)
        )
        adj16 = adj_mat[b].astype(BF16_NP).reshape(NT, P, OB, P)
        adj_r = np.ascontiguousarray(adj16.transpose(3, 0, 2, 1))  # [po, t, ob, ps]
        nodeT = np.ascontiguousarray(node_state[b].T).astype(BF16_NP)  # [j, s]
        in_maps.append({"edge_t": edge_t, "adj_r": adj_r, "nodeT": nodeT, "wT": wT})
    return in_maps


def kernel(node_state, edge_type_mat, adj_mat, W):
    nc = get_nc()
    in_maps = make_in_maps(node_state, edge_type_mat, adj_mat, W)
    res = run_bass_kernel_spmd(nc, in_maps, list(range(B)))
    # out is [p, t, i] per core -> [s, i] with s = t*P + p
    return np.stack(
        [res.results[b]["out"].transpose(1, 0, 2).reshape(N, D) for b in range(B)],
        axis=0,
    )


# revision 16
# speedup vs baseline: 1.0799x; 1.0799x over previous
"""Trainium2 Bass kernel for nn_MessagePassing (gnn_message_passing).

Math (per batch b):
    coef[s,e] = sum_o adj[s,o] * edge[s,o,e]
    v[s,e,i]  = sum_j W[e,i,j] * node[s,j]
    out[s,i]  = sum_e coef[s,e] * v[s,e,i]

Sharding: data parallel over the batch axis - core b handles batch b.

Host-side staging (per core):
  * edge  -> [t, o%128, o//128, e, s%128] bf16: o on SBUF partitions so the
    o-reduction runs on the PE; contiguous 2 MiB DMA per s-tile.
  * adj   -> [o%128, t, o//128, s%128] bf16 (same partition layout).
  * node  -> nodeT [j, s] bf16, W -> wT [j, e, i] bf16.
  * out   <- [p, t, i] f32, one contiguous DMA.

Engine assignment per s-tile (measured HW rates):
  * DVE: 8 2x-TT multiplies prod[o, e, s] = edge * adj (adj broadcast over
    the middle e dim; bf16-packed 2x mode, ~0.54 ns/elem) and one 2x-
    ineligible TT (PSUM operand) forming uT[j,e,s] = nodeT[j,s]*coef[s,e].
  * PE : the ENTIRE reduction: coefRow[1,(e,s)] = sum_o ones^T @ prod
    accumulated over the 8 o-blocks in PSUM, then 2 ones-matmuls that
    broadcast coefRow across partitions, then 8 PSUM-accumulated matmuls
    out[s,i] += uT_e^T @ W_e.
  * ACT: small copies only (coefRow PSUM->SBUF, out PSUM->SBUF).
  * Pool shares the DVE SBUF port - unused.
"""

import numpy as np
import ml_dtypes
from contextlib import ExitStack

import concourse.bass as bass
import concourse.bacc as bacc
import concourse.mybir as mybir
import concourse.tile as tile
from concourse.bass_utils import run_bass_kernel_spmd

B, N, D, E = 8, 1024, 128, 8
P = 128
NT = N // P  # 8 s-tiles per core
OB = N // P  # 8 o-blocks
HF = E * P // 2  # 512: half of the flattened (e, s) row

F32 = mybir.dt.float32
BF16 = mybir.dt.bfloat16
MUL = mybir.AluOpType.mult
COPY = mybir.ActivationFunctionType.Copy

BF16_NP = ml_dtypes.bfloat16


def build_nc():
    nc = bacc.Bacc("TRN2", target_bir_lowering=False, debug=False, num_devices=B)

    edge_d = nc.dram_tensor("edge_t", [NT, P, OB, E, P], BF16, kind="ExternalInput").ap()
    adj_d = nc.dram_tensor("adj_r", [P, NT, OB, P], BF16, kind="ExternalInput").ap()
    nodeT_d = nc.dram_tensor("nodeT", [D, N], BF16, kind="ExternalInput").ap()
    wT_d = nc.dram_tensor("wT", [D, E, D], BF16, kind="ExternalInput").ap()
    out_d = nc.dram_tensor("out", [P, NT, D], F32, kind="ExternalOutput").ap()

    with tile.TileContext(nc) as tc, ExitStack() as ctx:
        const_pool = ctx.enter_context(tc.tile_pool(name="const", bufs=1))
        edge_pool = ctx.enter_context(tc.tile_pool(name="edge", bufs=4))
        prod_pool = ctx.enter_context(tc.tile_pool(name="prod", bufs=2))
        work_pool = ctx.enter_context(tc.tile_pool(name="work", bufs=2))
        ps_row_pool = ctx.enter_context(tc.tile_pool(name="psr", bufs=2, space="PSUM"))
        ps_bc_pool = ctx.enter_context(tc.tile_pool(name="psb", bufs=1, space="PSUM"))
        ps_out_pool = ctx.enter_context(tc.tile_pool(name="pso", bufs=2, space="PSUM"))

        adj_all = const_pool.tile([P, NT, OB, P], BF16)
        nodeT = const_pool.tile([P, N], BF16)
        wT = const_pool.tile([P, E, D], BF16)
        ones_r = const_pool.tile([P, 1], BF16)  # reduce lhsT
        ones_b = const_pool.tile([1, P], BF16)  # broadcast lhsT
        acc_all = const_pool.tile([P, NT, D], F32)

        nc.vector.memset(ones_r[:], 1.0)
        nc.vector.memset(ones_b[:], 1.0)

        # Edge stream on the sync queue (tile 0 split for a fast start);
        # everything else on the scalar queue.
        def load_edge(t, split=False):
            et = edge_pool.tile([P, OB, E, P], BF16, tag="edge_t")
            if split:
                nc.sync.dma_start(et[:, 0:1, :, :], edge_d[t, :, 0:1])
                nc.sync.dma_start(et[:, 1:3, :, :], edge_d[t, :, 1:3])
                nc.sync.dma_start(et[:, 3:OB, :, :], edge_d[t, :, 3:OB])
            else:
                nc.sync.dma_start(et[:], edge_d[t])
            return et

        edge_tiles = {0: load_edge(0, split=True)}
        nc.scalar.dma_start(adj_all[:, 0, :, :], adj_d[:, 0, :, :])
        nc.scalar.dma_start(nodeT[:], nodeT_d)
        nc.scalar.dma_start(wT[:], wT_d)
        for t in range(1, NT):
            edge_tiles[t] = load_edge(t)
            nc.scalar.dma_start(adj_all[:, t, :, :], adj_d[:, t, :, :])

        for t in range(NT):
            edge_t = edge_tiles[t]

            # DVE: prod[o, e, s] = edge * adj (2x TT, adj bcast over e) and
            # PE: coefRow[1, (e,s)] += ones^T @ prod, per o-block.
            prod = prod_pool.tile([P, OB, E, P], BF16)
            rowA = ps_row_pool.tile([1, HF], F32, tag="rowA")
            rowB = ps_row_pool.tile([1, HF], F32, tag="rowB")
            rows = [rowA, rowB]
            for ob in range(OB):
                nc.vector.tensor_tensor(
                    out=prod[:, ob, :, :],
                    in0=edge_t[:, ob, :, :],
                    in1=adj_all[:, t, ob, :][:, None, :].broadcast_to((P, E, P)),
                    op=MUL,
                )
                flat = prod[:, ob, :, :].rearrange("p e s -> p (e s)")
                for h in range(2):
                    nc.tensor.matmul(
                        rows[h][:],
                        lhsT=ones_r[:],
                        rhs=flat[:, h * HF : (h + 1) * HF],
                        start=(ob == 0),
                        stop=(ob == OB - 1),
                    )

            # ACT: coefRow PSUM -> SBUF bf16
            coefRow = work_pool.tile([1, E * P], BF16)
            for h in range(2):
                nc.scalar.copy(coefRow[:, h * HF : (h + 1) * HF], rows[h][:])

            # PE: broadcast coefRow across 128 partitions
            bcA = ps_bc_pool.tile([P, HF], F32, tag="bcA")
            bcB = ps_bc_pool.tile([P, HF], F32, tag="bcB")
            coefB = [bcA, bcB]
            for h in range(2):
                nc.tensor.matmul(
                    coefB[h][:],
                    lhsT=ones_b[:],
                    rhs=coefRow[:, h * HF : (h + 1) * HF],
                    start=True,
                    stop=True,
                )

            # DVE: uT[j, e, s] = nodeT[j, s] * coef[s, e]
            uT = work_pool.tile([P, E, P], BF16)
            for h in range(2):
                nc.vector.tensor_tensor(
                    out=uT[:, h * 4 : (h + 1) * 4, :],
                    in0=nodeT[:, bass.ts(t, P)][:, None, :].broadcast_to((P, 4, P)),
                    in1=coefB[h][:].rearrange("p (e s) -> p e s", e=4),
                    op=MUL,
                )

            # PE: out[s, i] = sum_e uT_e^T @ W_e, accumulated in PSUM
            po = ps_out_pool.tile([P, D], F32, tag="po")
            for e in range(E):
                nc.tensor.matmul(
                    po[:], lhsT=uT[:, e, :], rhs=wT[:, e, :],
                    start=(e == 0), stop=(e == E - 1),
                )
            # ACT: out PSUM -> SBUF, then stream out per tile
            nc.scalar.copy(acc_all[:, t, :], po[:])
            nc.sync.dma_start(out_d[:, t, :], acc_all[:, t, :])

    nc.compile()
    return nc


_NC_CACHE = None


def get_nc():
    global _NC_CACHE
    if _NC_CACHE is None:
        _NC_CACHE = build_nc()
    return _NC_CACHE


def make_in_maps(node_state, edge_type_mat, adj_mat, W):
    node_state = np.asarray(node_state, dtype=np.float32)
    edge_type_mat = np.asarray(edge_type_mat, dtype=np.float32)
    adj_mat = np.asarray(adj_mat, dtype=np.float32)
    W = np.asarray(W, dtype=np.float32)

    wT = np.ascontiguousarray(W.transpose(2, 0, 1)).astype(BF16_NP)  # [j, e, i]
    in_maps = []
    for b in range(B):
        edge16 = edge_type_mat[b].astype(BF16_NP)  # [s, o, e]
        # [t, po, ob, e, ps]
        edge_t = np.ascontiguousarray(
            edge16.reshape(NT, P, OB, P, E).transpose(0, 3, 2, 4, 1)
        )
        adj16 = adj_mat[b].astype(BF16_NP).reshape(NT, P, OB, P)
        adj_r = np.ascontiguousarray(adj16.transpose(3, 0, 2, 1))  # [po, t, ob, ps]
        nodeT = np.ascontiguousarray(node_state[b].T).astype(BF16_NP)  # [j, s]
        in_maps.append({"edge_t": edge_t, "adj_r": adj_r, "nodeT": nodeT, "wT": wT})
    return in_maps


def kernel(node_state, edge_type_mat, adj_mat, W):
    nc = get_nc()
    in_maps = make_in_maps(node_state, edge_type_mat, adj_mat, W)
    res = run_bass_kernel_spmd(nc, in_maps, list(range(B)))
    # out is [p, t, i] per core -> [s, i] with s = t*P + p
    return np.stack(
        [res.results[b]["out"].transpose(1, 0, 2).reshape(N, D) for b in range(B)],
        axis=0,
    )


# revision 18
# speedup vs baseline: 1.1882x; 1.1003x over previous
"""Trainium2 Bass kernel for nn_MessagePassing (gnn_message_passing).

Math (per batch b):
    coef[s,e] = sum_o adj[s,o] * edge[s,o,e]
    v[s,e,i]  = sum_j W[e,i,j] * node[s,j]
    out[s,i]  = sum_e coef[s,e] * v[s,e,i]

Sharding: data parallel over the batch axis - core b handles batch b.

Host-side staging (per core):
  * edge  -> [t, o%128, o//128, e, s%128] bf16: o on SBUF partitions so the
    o-reduction runs on the PE; contiguous 2 MiB DMA per s-tile.
  * adj   -> [o%128, t, o//128, s%128] bf16 (same partition layout).
  * node  -> nodeT [j, s] bf16, W -> wT [j, e, i] bf16.
  * out   <- [p, t, i] f32, one contiguous DMA.

Engine assignment per s-tile (measured HW rates):
  * DVE: 8 2x-TT multiplies prod[o, e, s] = edge * adj (adj broadcast over
    the middle e dim; bf16-packed 2x mode, ~0.54 ns/elem) and one 2x-
    ineligible TT (PSUM operand) forming uT[j,e,s] = nodeT[j,s]*coef[s,e].
  * PE : the ENTIRE reduction: coefRow[1,(e,s)] = sum_o ones^T @ prod
    accumulated over the 8 o-blocks in PSUM, then 2 ones-matmuls that
    broadcast coefRow across partitions, then 8 PSUM-accumulated matmuls
    out[s,i] += uT_e^T @ W_e.
  * ACT: small copies only (coefRow PSUM->SBUF, out PSUM->SBUF).
  * Pool shares the DVE SBUF port - unused.
"""

import numpy as np
import ml_dtypes
from contextlib import ExitStack

import concourse.bass as bass
import concourse.bacc as bacc
import concourse.mybir as mybir
import concourse.tile as tile
from concourse.bass_utils import run_bass_kernel_spmd

B, N, D, E = 8, 1024, 128, 8
P = 128
NT = N // P  # 8 s-tiles per core
OB = N // P  # 8 o-blocks
HF = E * P // 2  # 512: half of the flattened (e, s) row

F32 = mybir.dt.float32
BF16 = mybir.dt.bfloat16
MUL = mybir.AluOpType.mult
COPY = mybir.ActivationFunctionType.Copy

BF16_NP = ml_dtypes.bfloat16


def build_nc():
    nc = bacc.Bacc("TRN2", target_bir_lowering=False, debug=False, num_devices=B)

    edge_d = nc.dram_tensor("edge_t", [NT, P, OB, E, P], BF16, kind="ExternalInput").ap()
    adj_d = nc.dram_tensor("adj_r", [P, NT, OB, P], BF16, kind="ExternalInput").ap()
    nodeT_d = nc.dram_tensor("nodeT", [D, N], BF16, kind="ExternalInput").ap()
    wT_d = nc.dram_tensor("wT", [D, E, D], BF16, kind="ExternalInput").ap()
    out_d = nc.dram_tensor("out", [P, NT, D], F32, kind="ExternalOutput").ap()

    with tile.TileContext(nc) as tc, ExitStack() as ctx:
        const_pool = ctx.enter_context(tc.tile_pool(name="const", bufs=1))
        edge_pool = ctx.enter_context(tc.tile_pool(name="edge", bufs=4))
        prod_pool = ctx.enter_context(tc.tile_pool(name="prod", bufs=2))
        work_pool = ctx.enter_context(tc.tile_pool(name="work", bufs=2))
        ps_row_pool = ctx.enter_context(tc.tile_pool(name="psr", bufs=2, space="PSUM"))
        ps_bc_pool = ctx.enter_context(tc.tile_pool(name="psb", bufs=1, space="PSUM"))
        ps_out_pool = ctx.enter_context(tc.tile_pool(name="pso", bufs=2, space="PSUM"))

        adj_all = const_pool.tile([P, NT, OB, P], BF16)
        nodeT = const_pool.tile([P, N], BF16)
        wT = const_pool.tile([P, E, D], BF16)
        ones_r = const_pool.tile([P, 1], BF16)  # reduce lhsT
        ones_b = const_pool.tile([1, P], BF16)  # broadcast lhsT
        acc_all = const_pool.tile([P, NT, D], F32)

        nc.vector.memset(ones_r[:], 1.0)
        nc.vector.memset(ones_b[:], 1.0)

        # Edge stream on the sync queue (tile 0 split for a fast start);
        # everything else on the scalar queue.
        def load_edge(t, split=False):
            et = edge_pool.tile([P, OB, E, P], BF16, tag="edge_t")
            if split:
                nc.sync.dma_start(et[:, 0:1, :, :], edge_d[t, :, 0:1])
                nc.sync.dma_start(et[:, 1:3, :, :], edge_d[t, :, 1:3])
                nc.sync.dma_start(et[:, 3:OB, :, :], edge_d[t, :, 3:OB])
            else:
                nc.sync.dma_start(et[:], edge_d[t])
            return et

        edge_tiles = {0: load_edge(0, split=True)}
        nc.scalar.dma_start(adj_all[:, 0, :, :], adj_d[:, 0, :, :])
        nc.scalar.dma_start(nodeT[:], nodeT_d)
        nc.scalar.dma_start(wT[:], wT_d)
        for t in range(1, NT):
            edge_tiles[t] = load_edge(t)
            nc.scalar.dma_start(adj_all[:, t, :, :], adj_d[:, t, :, :])

        for t in range(NT):
            edge_t = edge_tiles[t]

            # DVE: prod[o, e, s] = edge * adj (2x TT, adj bcast over e) and
            # PE: coefRow[1, (e,s)] += ones^T @ prod, per o-block.
            prod = prod_pool.tile([P, OB, E, P], BF16)
            rowA = ps_row_pool.tile([1, HF], F32, tag="rowA")
            rowB = ps_row_pool.tile([1, HF], F32, tag="rowB")
            rows = [rowA, rowB]
            for ob in range(OB):
                nc.vector.tensor_tensor(
                    out=prod[:, ob, :, :],
                    in0=edge_t[:, ob, :, :],
                    in1=adj_all[:, t, ob, :][:, None, :].broadcast_to((P, E, P)),
                    op=MUL,
                )
                flat = prod[:, ob, :, :].rearrange("p e s -> p (e s)")
                for h in range(2):
                    nc.tensor.matmul(
                        rows[h][:],
                        lhsT=ones_r[:],
                        rhs=flat[:, h * HF : (h + 1) * HF],
                        start=(ob == 0),
                        stop=(ob == OB - 1),
                    )

            # ACT: coefRow PSUM -> SBUF bf16
            coefRow = work_pool.tile([1, E * P], BF16)
            for h in range(2):
                nc.scalar.copy(coefRow[:, h * HF : (h + 1) * HF], rows[h][:])

            # PE: broadcast coefRow across 128 partitions (one 2-bank tile)
            coefB = ps_bc_pool.tile([P, 2, HF], F32, tag="cB")
            for h in range(2):
                nc.tensor.matmul(
                    coefB[:, h, :],
                    lhsT=ones_b[:],
                    rhs=coefRow[:, h * HF : (h + 1) * HF],
                    start=True,
                    stop=True,
                )
            # ACT: coefB PSUM -> SBUF bf16 so the uT multiply runs in 2x mode
            cb_s = work_pool.tile([P, E, P], BF16)
            nc.scalar.copy(cb_s[:], coefB[:].rearrange("p h (e s) -> p (h e) s", e=E // 2))

            # DVE: uT[j, e, s] = nodeT[j, s] * coef[s, e]  (single 2x TT)
            uT = work_pool.tile([P, E, P], BF16)
            nc.vector.tensor_tensor(
                out=uT[:],
                in0=nodeT[:, bass.ts(t, P)][:, None, :].broadcast_to((P, E, P)),
                in1=cb_s[:],
                op=MUL,
            )

            # PE: out[s, i] = sum_e uT_e^T @ W_e, accumulated in PSUM
            po = ps_out_pool.tile([P, D], F32, tag="po")
            for e in range(E):
                nc.tensor.matmul(
                    po[:], lhsT=uT[:, e, :], rhs=wT[:, e, :],
                    start=(e == 0), stop=(e == E - 1),
                )
            # ACT: out PSUM -> SBUF, then stream out per tile
            nc.scalar.copy(acc_all[:, t, :], po[:])
            nc.scalar.dma_start(out_d[:, t, :], acc_all[:, t, :])

    nc.compile()
    return nc


_NC_CACHE = None


def get_nc():
    global _NC_CACHE
    if _NC_CACHE is None:
        _NC_CACHE = build_nc()
    return _NC_CACHE


def make_in_maps(node_state, edge_type_mat, adj_mat, W):
    node_state = np.asarray(node_state, dtype=np.float32)
    edge_type_mat = np.asarray(edge_type_mat, dtype=np.float32)
    adj_mat = np.asarray(adj_mat, dtype=np.float32)
    W = np.asarray(W, dtype=np.float32)

    wT = np.ascontiguousarray(W.transpose(2, 0, 1)).astype(BF16_NP)  # [j, e, i]
    in_maps = []
    for b in range(B):
        edge16 = edge_type_mat[b].astype(BF16_NP)  # [s, o, e]
        # [t, po, ob, e, ps]
        edge_t = np.ascontiguousarray(
            edge16.reshape(NT, P, OB, P, E).transpose(0, 3, 2, 4, 1)
        )
        adj16 = adj_mat[b].astype(BF16_NP).reshape(NT, P, OB, P)
        adj_r = np.ascontiguousarray(adj16.transpose(3, 0, 2, 1))  # [po, t, ob, ps]
        nodeT = np.ascontiguousarray(node_state[b].T).astype(BF16_NP)  # [j, s]
        in_maps.append({"edge_t": edge_t, "adj_r": adj_r, "nodeT": nodeT, "wT": wT})
    return in_maps


def kernel(node_state, edge_type_mat, adj_mat, W):
    nc = get_nc()
    in_maps = make_in_maps(node_state, edge_type_mat, adj_mat, W)
    res = run_bass_kernel_spmd(nc, in_maps, list(range(B)))
    # out is [p, t, i] per core -> [s, i] with s = t*P + p
    return np.stack(
        [res.results[b]["out"].transpose(1, 0, 2).reshape(N, D) for b in range(B)],
        axis=0,
    )
